# revision 54
# baseline (speedup 1.0000x reference)
"""GPTQ-style grouped-dequant linear on 8 Trainium2 cores.

out[m,n] = sum_k A[m,k] * (q[n,k] - zeros[n,k//128]) * scales[n,k//128] + bias[n]
M=2048, K=4096, N=4096, group=128.

Sharding: column-parallel — qweight/scales/zeros/bias split along N (512/core),
A replicated. Host does layout permutes + dtype casts only: A pre-cast to bf16
(same rounding the device matmul path applies anyway), q repacked to uint8,
scales/zeros pre-broadcast along the 128 k-partitions (pure replication), and
the whole W side (q/scales/zeros) byte-packed into one chunked tensor so each
4-group chunk is a single DMA (bitcast views recover the bf16 operands).

Per core: dequant is two DVE tensor_tensor ops per k-group producing bf16 W^T
tiles in [k,n] layout, streaming through a rotating SBUF window. The only PE
work is the 512 productive 128x128x512 matmuls (16 m-tiles x 32 k-groups).
All eight lead m-tiles load their A as half-tiles (groups 0-15 first) so
eight accumulation chains go live off the first chunks — the PE saturates as
soon as the ramp allows and stays dense; a short dummy-MM spin holds the HAM
clock gate open through the initial DMA latency. Every transfer rides the
sync queue in exact consumption order (delivery order == issue order), so
nothing can steal warmup bandwidth. Bias is folded into the PSUM->SBUF
eviction (DVE add); output is bf16, upcast on host.
"""

import numpy as np
import ml_dtypes

import concourse.bass as bass
import concourse.mybir as mybir
import concourse.tile as tile
from concourse import bacc
from concourse.bass_utils import run_bass_kernel_spmd

P = 128
M, K, N = 2048, 4096, 4096
NCORES = 8
NS = N // NCORES          # 512 out-features per core
G = K // P                # 32 groups (group_size == P == 128)
MT = M // P               # 16 output row tiles

NLEAD = 8                 # lead m-tiles resident in PSUM during warmup
JOIN_AT = {0: 0, 1: 0, 2: 2, 3: 2, 4: 6, 5: 6, 6: 8, 7: 8}
CH = 4                    # groups per packed W chunk (one DMA each)
CHB = CH * NS * 5         # chunk bytes/partition: q u8 + s bf16 + z bf16
NDUMMY = 16               # warmup matmuls holding the HAM clock gate open

_cached = None


def _build():
    nc = bacc.Bacc("TRN2", target_bir_lowering=False, debug=False,
                   num_devices=NCORES)
    at = nc.dram_tensor("AT4", [MT, P, G, P], mybir.dt.bfloat16,
                        kind="ExternalInput")
    # lead A as eight contiguous 1MB slabs (lead-pair x g-half): [p, h, t, j]
    aw = nc.dram_tensor("AW", [4, 2, P, G // 2, 2, P], mybir.dt.bfloat16,
                        kind="ExternalInput")
    wp = nc.dram_tensor("wpk", [P, G // CH, CHB], mybir.dt.uint8,
                        kind="ExternalInput")
    bi = nc.dram_tensor("brep", [P, NS], mybir.dt.float32,
                        kind="ExternalInput")
    out = nc.dram_tensor("out", [M, NS], mybir.dt.bfloat16,
                         kind="ExternalOutput")

    bf16, f32 = mybir.dt.bfloat16, mybir.dt.float32
    NCH = G // CH
    H = G // 2

    with tile.TileContext(nc) as tc:
        with (
            tc.tile_pool(name="const", bufs=1) as const,
            tc.tile_pool(name="wpool", bufs=3) as wpool,
            tc.tile_pool(name="tmp", bufs=3) as tmpp,
            tc.tile_pool(name="wt", bufs=1) as wtp,
            tc.tile_pool(name="apool", bufs=4) as apool,
            tc.tile_pool(name="mpsum", bufs=8, space="PSUM") as mpsum,
            tc.tile_pool(name="opool", bufs=3) as opool,
        ):
            bias_r = const.tile([P, NS], f32, tag="bias_r")
            scratch = const.tile([P, NS], bf16, tag="scratch")
            nc.gpsimd.memset(scratch, 0.0)
            wr = wp.ap()
            atr = at.ap()  # [MT, P, G, P], per-partition contiguous

            def load_wpk(c):
                wc = wpool.tile([P, CHB], mybir.dt.uint8, tag="wc", bufs=3)
                nc.sync.dma_start(out=wc[:], in_=wr[:, c, :])
                return wc

            # sync queue, delivery order == consumption order: W chunks
            # interleaved with the four 2MB lead-A quadrant slabs (each one
            # contiguous DMA covering 4 leads x 16 groups)
            awr = aw.ap()
            awt = {}
            for tb in range(4):
                for gb in range(2):
                    awt[tb, gb] = const.tile([P, H, 2, P], bf16,
                                             name=f"aw{tb}{gb}")

            def lead_lhsT(mt, g):
                return awt[mt // 2, g // H][:, g % H, mt % 2, :]

            def load_aw(tb, gb):
                nc.sync.dma_start(out=awt[tb, gb][:], in_=awr[tb, gb])

            wcs = [None] * NCH
            wcs[0] = load_wpk(0)
            load_aw(0, 0)
            wcs[1] = load_wpk(1)
            load_aw(1, 0)
            wcs[2] = load_wpk(2)
            load_aw(2, 0)
            load_aw(3, 0)
            wcs[3] = load_wpk(3)
            load_aw(0, 1)
            load_aw(1, 1)
            wcs[4] = load_wpk(4)
            wcs[5] = load_wpk(5)
            load_aw(2, 1)
            load_aw(3, 1)
            wcs[6] = load_wpk(6)
            wcs[7] = load_wpk(7)
            nc.sync.dma_start(out=bias_r[:], in_=bi.ap()[:])
            # phase-2 A tiles, position-gated on the same queue (and by the
            # apool ring) so they never steal warmup bandwidth
            pre = []
            for mt in range(NLEAD, MT):
                ab = apool.tile([P, G, P], bf16)
                nc.sync.dma_start(out=ab[:], in_=atr[mt, :, :, :])
                pre.append(ab)

            def new_ps():
                ps = mpsum.tile([P, NS], f32)
                return ps

            # warmup spin: hold the HAM clock gate open while DMA streams in
            # (continued by per-group filler matmuls below until lead7 joins)
            dummy_ps = new_ps()
            for i in range(NDUMMY):
                nc.tensor.matmul(dummy_ps[:], scratch[:, :P], scratch[:],
                                 start=(i == 0), stop=False)

            def spin(n, last=False):
                for i in range(n):
                    nc.tensor.matmul(dummy_ps[:], scratch[:, :P], scratch[:],
                                     start=False, stop=(last and i == n - 1))

            lead_ps = [new_ps() for _ in range(NLEAD)]

            def finish(mt, ps):
                ob = opool.tile([P, NS], bf16)
                nc.vector.tensor_tensor(ob[:], ps[:], bias_r[:],
                                        mybir.AluOpType.add)
                nc.sync.dma_start(out=out.ap()[mt * P:(mt + 1) * P, :],
                                  in_=ob[:])

            # Phase 1: dequant each k-group on DVE (operands are bitcast
            # views into the packed chunk), immediately consumed by the lead
            # tiles' PSUM accumulation chains.
            wts = []
            for g in range(G):
                wc = wcs[g // CH]
                j = g % CH
                q_ap = wc[:, j * NS:(j + 1) * NS]
                s_ap = wc[:, CH * NS + j * 2 * NS:
                          CH * NS + (j + 1) * 2 * NS].bitcast(bf16)
                z_ap = wc[:, 3 * CH * NS + j * 2 * NS:
                          3 * CH * NS + (j + 1) * 2 * NS].bitcast(bf16)
                tmp = tmpp.tile([P, NS], bf16)
                nc.vector.tensor_tensor(tmp[:], q_ap, z_ap,
                                        mybir.AluOpType.subtract)
                wt = wtp.tile([P, NS], bf16, tag=f"wt{g}")
                nc.vector.tensor_tensor(wt[:], tmp[:], s_ap,
                                        mybir.AluOpType.mult)
                wts.append(wt)
                for mt in range(NLEAD):
                    if JOIN_AT[mt] == g:
                        for gc in range(g + 1):  # catch-up burst
                            nc.tensor.matmul(lead_ps[mt][:],
                                             lead_lhsT(mt, gc), wts[gc][:],
                                             start=(gc == 0),
                                             stop=(gc == G - 1))
                    elif JOIN_AT[mt] < g:
                        nc.tensor.matmul(lead_ps[mt][:], lead_lhsT(mt, g),
                                         wt[:], start=False,
                                         stop=(g == G - 1))
                if g < JOIN_AT[NLEAD - 1]:
                    # filler: keep the HAM clock gate open through the
                    # bandwidth-bound ramp (PE would idle here anyway)
                    spin(4 if g < 4 else 3,
                         last=(g == JOIN_AT[NLEAD - 1] - 1))

            for mt in range(NLEAD):
                finish(mt, lead_ps[mt])

            # Phase 2: remaining output tiles, dense back-to-back matmuls
            for mt in range(NLEAD, MT):
                ab = pre[mt - NLEAD]
                ps = new_ps()
                for g in range(G):
                    nc.tensor.matmul(ps[:], ab[:, g, :], wts[g][:],
                                     start=(g == 0), stop=(g == G - 1))
                finish(mt, ps)

    nc.compile()
    return nc


def _prep_inputs(A, qweight, scales, zeros, bias):
    # AT4[mt, p, g, j] = A[mt*128+j, g*128+p]  (layout permute + bf16 cast)
    at4 = np.ascontiguousarray(
        A.reshape(MT, P, G, P).transpose(0, 3, 2, 1).astype(ml_dtypes.bfloat16))
    # lead A pair slabs: AW[tb, gb, p, h, t, j] = at4[2*tb+t, p, 16*gb+h, j]
    H = G // 2
    awn = np.ascontiguousarray(
        at4[:NLEAD].reshape(4, 2, P, 2, H, P).transpose(0, 3, 2, 4, 1, 5))
    NCH = G // CH
    in_maps = []
    for c in range(NCORES):
        r = slice(c * NS, (c + 1) * NS)
        # q4[p, g, n] = q[n, g*128+p]
        q4 = np.ascontiguousarray(
            qweight[r].astype(np.uint8).T.reshape(G, P, NS).transpose(1, 0, 2))
        # scales/zeros pre-broadcast across the 128 k-partitions (replication)
        srep = np.broadcast_to(
            scales[r].T.astype(ml_dtypes.bfloat16)[None, :, :], (P, G, NS))
        zrep = np.broadcast_to(
            zeros[r].T.astype(ml_dtypes.bfloat16)[None, :, :], (P, G, NS))
        # byte-pack per 4-group chunk: [q u8 | s bf16 | z bf16]
        qb = q4.reshape(P, NCH, CH * NS)
        sb = np.ascontiguousarray(srep).view(np.uint8).reshape(
            P, NCH, CH * NS * 2)
        zb = np.ascontiguousarray(zrep).view(np.uint8).reshape(
            P, NCH, CH * NS * 2)
        wpk = np.ascontiguousarray(np.concatenate([qb, sb, zb], axis=2))
        brep = np.ascontiguousarray(np.broadcast_to(
            bias[r].astype(np.float32)[None, :], (P, NS)))
        in_maps.append({"AT4": at4, "AW": awn, "wpk": wpk, "brep": brep})
    return in_maps


def run(inputs, **spmd_kwargs):
    global _cached
    if _cached is None:
        _cached = _build()
    in_maps = _prep_inputs(**inputs)
    res = run_bass_kernel_spmd(_cached, in_maps, list(range(NCORES)),
                               **spmd_kwargs)
    outp = np.concatenate(
        [res.results[c]["out"].astype(np.float32) for c in range(NCORES)],
        axis=1)
    return outp, res


def kernel(**inputs):
    return run(inputs)[0]


# revision 55
# speedup vs baseline: 1.1899x; 1.1899x over previous
"""GPTQ-style grouped-dequant linear on 8 Trainium2 cores.

out[m,n] = sum_k A[m,k] * (q[n,k] - zeros[n,k//128]) * scales[n,k//128] + bias[n]
M=2048, K=4096, N=4096, group=128.

Sharding: column-parallel — qweight/scales/zeros/bias split along N (512/core),
A replicated. Host does layout permutes + dtype casts only: A pre-cast to bf16
(same rounding the device matmul path applies anyway), q repacked to uint8,
scales/zeros pre-broadcast along the 128 k-partitions (pure replication), and
the whole W side (q/scales/zeros) byte-packed into one chunked tensor so each
4-group chunk is a single DMA (bitcast views recover the bf16 operands).

Per core: dequant is two DVE tensor_tensor ops per k-group producing bf16 W^T
tiles in [k,n] layout, streaming through a rotating SBUF window. The only PE
work is the 512 productive 128x128x512 matmuls (16 m-tiles x 32 k-groups).
All eight lead m-tiles load their A as half-tiles (groups 0-15 first) so
eight accumulation chains go live off the first chunks — the PE saturates as
soon as the ramp allows and stays dense; a short dummy-MM spin holds the HAM
clock gate open through the initial DMA latency. Every transfer rides the
sync queue in exact consumption order (delivery order == issue order), so
nothing can steal warmup bandwidth. Bias is folded into the PSUM->SBUF
eviction (DVE add); output is bf16, upcast on host.
"""

import numpy as np
import ml_dtypes

import concourse.bass as bass
import concourse.mybir as mybir
import concourse.tile as tile
from concourse import bacc
from concourse.bass_utils import run_bass_kernel_spmd

P = 128
M, K, N = 2048, 4096, 4096
NCORES = 8
NS = N // NCORES          # 512 out-features per core
G = K // P                # 32 groups (group_size == P == 128)
MT = M // P               # 16 output row tiles

NLEAD = 8                 # lead m-tiles resident in PSUM during warmup
JOIN_AT = {0: 0, 1: 0, 2: 2, 3: 2, 4: 6, 5: 6, 6: 8, 7: 8}
CH = 4                    # groups per packed W chunk (one DMA each)
CHB = CH * NS * 5         # chunk bytes/partition: q u8 + s bf16 + z bf16
NDUMMY = 16               # warmup matmuls holding the HAM clock gate open

_cached = None


def _build():
    nc = bacc.Bacc("TRN2", target_bir_lowering=False, debug=False,
                   num_devices=NCORES)
    at = nc.dram_tensor("AT4", [MT, P, G, P], mybir.dt.bfloat16,
                        kind="ExternalInput")
    # lead A as eight contiguous 1MB slabs (lead-pair x g-half): [p, h, t, j]
    aw = nc.dram_tensor("AW", [4, 2, P, G // 2, 2, P], mybir.dt.bfloat16,
                        kind="ExternalInput")
    wp = nc.dram_tensor("wpk", [P, G // CH, CHB], mybir.dt.uint8,
                        kind="ExternalInput")
    bi = nc.dram_tensor("brep", [P, NS], mybir.dt.float32,
                        kind="ExternalInput")
    out = nc.dram_tensor("out", [M, NS], mybir.dt.bfloat16,
                         kind="ExternalOutput")

    bf16, f32 = mybir.dt.bfloat16, mybir.dt.float32
    NCH = G // CH
    H = G // 2

    with tile.TileContext(nc) as tc:
        with (
            tc.tile_pool(name="const", bufs=1) as const,
            tc.tile_pool(name="wpool", bufs=3) as wpool,
            tc.tile_pool(name="tmp", bufs=3) as tmpp,
            tc.tile_pool(name="wt", bufs=1) as wtp,
            tc.tile_pool(name="apool", bufs=4) as apool,
            tc.tile_pool(name="mpsum", bufs=8, space="PSUM") as mpsum,
            tc.tile_pool(name="opool", bufs=3) as opool,
        ):
            bias_r = const.tile([P, NS], f32, tag="bias_r")
            scratch = const.tile([P, NS], bf16, tag="scratch")
            nc.gpsimd.memset(scratch, 0.0)
            wr = wp.ap()
            atr = at.ap()  # [MT, P, G, P], per-partition contiguous

            def load_wpk(c):
                wc = wpool.tile([P, CHB], mybir.dt.uint8, tag="wc", bufs=3)
                nc.sync.dma_start(out=wc[:], in_=wr[:, c, :])
                return wc

            # sync queue, delivery order == consumption order: W chunks
            # interleaved with the four 2MB lead-A quadrant slabs (each one
            # contiguous DMA covering 4 leads x 16 groups)
            awr = aw.ap()
            awt = {}
            for tb in range(4):
                for gb in range(2):
                    awt[tb, gb] = const.tile([P, H, 2, P], bf16,
                                             name=f"aw{tb}{gb}")

            def lead_lhsT(mt, g):
                return awt[mt // 2, g // H][:, g % H, mt % 2, :]

            def load_aw(tb, gb):
                nc.sync.dma_start(out=awt[tb, gb][:], in_=awr[tb, gb])

            wcs = [None] * NCH
            wcs[0] = load_wpk(0)
            load_aw(0, 0)
            wcs[1] = load_wpk(1)
            load_aw(1, 0)
            wcs[2] = load_wpk(2)
            load_aw(2, 0)
            load_aw(3, 0)
            wcs[3] = load_wpk(3)
            load_aw(0, 1)
            load_aw(1, 1)
            wcs[4] = load_wpk(4)
            wcs[5] = load_wpk(5)
            load_aw(2, 1)
            load_aw(3, 1)
            wcs[6] = load_wpk(6)
            wcs[7] = load_wpk(7)
            nc.sync.dma_start(out=bias_r[:], in_=bi.ap()[:])
            # phase-2 A tiles, position-gated on the same queue (and by the
            # apool ring) so they never steal warmup bandwidth
            pre = []
            for mt in range(NLEAD, MT):
                ab = apool.tile([P, G, P], bf16)
                nc.sync.dma_start(out=ab[:], in_=atr[mt, :, :, :])
                pre.append(ab)

            def new_ps():
                ps = mpsum.tile([P, NS], f32)
                return ps

            # warmup spin: hold the HAM clock gate open while DMA streams in
            # (continued by per-group filler matmuls below until lead7 joins)
            dummy_ps = new_ps()
            for i in range(NDUMMY):
                nc.tensor.matmul(dummy_ps[:], scratch[:, :P], scratch[:],
                                 start=(i == 0), stop=False)

            def spin(n, last=False):
                for i in range(n):
                    nc.tensor.matmul(dummy_ps[:], scratch[:, :P], scratch[:],
                                     start=False, stop=(last and i == n - 1))

            lead_ps = [new_ps() for _ in range(NLEAD)]

            def finish(mt, ps):
                ob = opool.tile([P, NS], bf16)
                nc.vector.tensor_tensor(ob[:], ps[:], bias_r[:],
                                        mybir.AluOpType.add)
                nc.sync.dma_start(out=out.ap()[mt * P:(mt + 1) * P, :],
                                  in_=ob[:])

            # Phase 1: dequant each k-group on DVE (operands are bitcast
            # views into the packed chunk), immediately consumed by the lead
            # tiles' PSUM accumulation chains.
            wts = []
            for g in range(G):
                wc = wcs[g // CH]
                j = g % CH
                q_ap = wc[:, j * NS:(j + 1) * NS]
                s_ap = wc[:, CH * NS + j * 2 * NS:
                          CH * NS + (j + 1) * 2 * NS].bitcast(bf16)
                z_ap = wc[:, 3 * CH * NS + j * 2 * NS:
                          3 * CH * NS + (j + 1) * 2 * NS].bitcast(bf16)
                tmp = tmpp.tile([P, NS], bf16)
                nc.vector.tensor_tensor(tmp[:], q_ap, z_ap,
                                        mybir.AluOpType.subtract)
                wt = wtp.tile([P, NS], bf16, tag=f"wt{g}")
                nc.vector.tensor_tensor(wt[:], tmp[:], s_ap,
                                        mybir.AluOpType.mult)
                wts.append(wt)
                for mt in range(NLEAD):
                    if JOIN_AT[mt] == g:
                        for gc in range(g + 1):  # catch-up burst
                            nc.tensor.matmul(lead_ps[mt][:],
                                             lead_lhsT(mt, gc), wts[gc][:],
                                             start=(gc == 0),
                                             stop=(gc == G - 1))
                    elif JOIN_AT[mt] < g:
                        nc.tensor.matmul(lead_ps[mt][:], lead_lhsT(mt, g),
                                         wt[:], start=False,
                                         stop=(g == G - 1))
                if g < JOIN_AT[NLEAD - 1]:
                    # filler: keep the HAM clock gate open through the
                    # bandwidth-bound ramp (PE would idle here anyway)
                    spin(3 if g < 4 else 2,
                         last=(g == JOIN_AT[NLEAD - 1] - 1))

            for mt in range(NLEAD):
                finish(mt, lead_ps[mt])

            # Phase 2: remaining output tiles, dense back-to-back matmuls
            for mt in range(NLEAD, MT):
                ab = pre[mt - NLEAD]
                ps = new_ps()
                for g in range(G):
                    nc.tensor.matmul(ps[:], ab[:, g, :], wts[g][:],
                                     start=(g == 0), stop=(g == G - 1))
                finish(mt, ps)

    nc.compile()
    return nc


def _prep_inputs(A, qweight, scales, zeros, bias):
    # AT4[mt, p, g, j] = A[mt*128+j, g*128+p]  (layout permute + bf16 cast)
    at4 = np.ascontiguousarray(
        A.reshape(MT, P, G, P).transpose(0, 3, 2, 1).astype(ml_dtypes.bfloat16))
    # lead A pair slabs: AW[tb, gb, p, h, t, j] = at4[2*tb+t, p, 16*gb+h, j]
    H = G // 2
    awn = np.ascontiguousarray(
        at4[:NLEAD].reshape(4, 2, P, 2, H, P).transpose(0, 3, 2, 4, 1, 5))
    NCH = G // CH
    in_maps = []
    for c in range(NCORES):
        r = slice(c * NS, (c + 1) * NS)
        # q4[p, g, n] = q[n, g*128+p]
        q4 = np.ascontiguousarray(
            qweight[r].astype(np.uint8).T.reshape(G, P, NS).transpose(1, 0, 2))
        # scales/zeros pre-broadcast across the 128 k-partitions (replication)
        srep = np.broadcast_to(
            scales[r].T.astype(ml_dtypes.bfloat16)[None, :, :], (P, G, NS))
        zrep = np.broadcast_to(
            zeros[r].T.astype(ml_dtypes.bfloat16)[None, :, :], (P, G, NS))
        # byte-pack per 4-group chunk: [q u8 | s bf16 | z bf16]
        qb = q4.reshape(P, NCH, CH * NS)
        sb = np.ascontiguousarray(srep).view(np.uint8).reshape(
            P, NCH, CH * NS * 2)
        zb = np.ascontiguousarray(zrep).view(np.uint8).reshape(
            P, NCH, CH * NS * 2)
        wpk = np.ascontiguousarray(np.concatenate([qb, sb, zb], axis=2))
        brep = np.ascontiguousarray(np.broadcast_to(
            bias[r].astype(np.float32)[None, :], (P, NS)))
        in_maps.append({"AT4": at4, "AW": awn, "wpk": wpk, "brep": brep})
    return in_maps


def run(inputs, **spmd_kwargs):
    global _cached
    if _cached is None:
        _cached = _build()
    in_maps = _prep_inputs(**inputs)
    res = run_bass_kernel_spmd(_cached, in_maps, list(range(NCORES)),
                               **spmd_kwargs)
    outp = np.concatenate(
        [res.results[c]["out"].astype(np.float32) for c in range(NCORES)],
        axis=1)
    return outp, res


def kernel(**inputs):
    return run(inputs)[0]
